# revision 53
# baseline (speedup 1.0000x reference)
"""Local (windowed) self-attention Trainium2 kernel.

Model (reference): LayerNorm -> per-window (W=1024) multi-head attention
(H=8 heads, K=32 head dim) -> output projection -> residual add.
Shapes: x [B=2, T=8192, C=512]; 16 independent windows of 1024 tokens.

Distribution: 16 windows / 8 cores = 2 windows per core (data parallel over
the B*n_chunks axis), QKV/O projection weights replicated, no collectives.

Per-core program (Tile framework, fully unrolled, bf16 matmuls / fp32 PSUM).
Engine budget per core (cost model): PE ~140us of streamed matmul columns
(serial floor — tile_position row/col packing gives NO concurrency on this
stack, measured), ScalarE ~134us (128 Exp ops of [128,1024], the pipeline
pacer), DVE ~75us, GpSimd ~15us, so the structure aims everything at
keeping ScalarE 100% fed and hiding all other work under it:

  Head: x DMAs first (the SP HWDGE queue issues ~1 DMA/650ns and is the
    head bottleneck; weights are packed into 3 DMAs total), LN(w0) in two
    4-tile half-batches so the first zT transposes / K,Q projections start
    after only half the window: per tile bn_stats -> batched
    rstd = 1/sqrt(var+eps) on DVE (quake bit-trick + 2 Newton steps; keeps
    ScalarE exp-only => ONE act-table load) -> z-write on GpSimd (fp32
    sbuf->bf16) -> z bounced through DRAM -> half-window DMA transposes to
    zT [C, tok].
  Attention (per window, 64 iterations of (q-tile 512, head pair,
    s-chunk 128)): scoresT [s, q] per head (contraction = head dim 32, row
    tile_position 32g) into double-buffered 2-bank PSUM; ONE ScalarE Exp
    [128, 1024] (scale folded) -> bf16 expT; ONE M=64 matmul per head
    (lhsT = [V_h | ones], col tile_position 0/64) accumulates attn@V and
    the softmax denominator over the 8 s-chunks; normalize via DVE
    reciprocal+mul. Scores for iter i+1 are emitted before A*V of iter i.
  Overlap: LN(w1) (x DMAs pre-issued; stats/z injected), QKV(w1), and
    outproj(w0)/half of outproj(w1) are all injected into the attention
    iteration stream at deadline-derived slots, so both windows' LN/proj
    work hides under the exp-paced pipeline.

Host-side prep (constant folding only): shard windows, fold LN gamma/beta
into projection weights/biases, fold bv through attention (softmax rows sum
to 1) into the output bias, cast weights to bf16, pack q/k/v weights and
biases for single-DMA loads. The runtime bias adds are skipped when the
folded bo is all-zero (it is for the spec's zero fills; a has_bo program
variant handles the general case).

Notes from tuning (measured on 8 axon NeuronCores):
  - tile_position row/col tiling gave no MM concurrency here (row2/row4
    micro-bench: 237-253ns per N=512 MM vs 193ns serial), so PE time is
    the serial sum of streamed columns; don't count on packing tricks.
  - A Schraudolph bit-exp on DVE (tensor_scalar fp32->int16, bf16-bit
    aliased) is numerically fine (2.3e-3 end-to-end worst case) but ~5x
    slower than ScalarE on real HW despite the cost model liking it
    (dve_exp_mod flag, default off).
  - Repeat-loop (reps>1) timing requires Pool in For_i hint_engines or
    the loop serializes; the graded single-shot path has no loop.

Measured: cost-model single-shot 200.1us (baseline build: 209.7us);
repeat-loop slope read 162us vs the baseline's 222us in a stable machine
window (the box later drifted +-20%, see test.py's paired timing).
Output absmax relative error 1.40e-3 vs the fp32 reference.
"""

import numpy as np
import ml_dtypes

import concourse.bass as bass
import concourse.tile as tile
from concourse import bacc, mybir
from concourse.bass_utils import run_bass_kernel_spmd

F32 = mybir.dt.float32
BF16 = mybir.dt.bfloat16

B, T, C, H, K = 2, 8192, 512, 8, 32
W = 1024
HK = H * K              # 256
N_CORES = 8
NW = (B * T) // W       # 16 windows
WPC = NW // N_CORES     # 2 windows per core
EPS = 1e-5
SCALE = 1.0 / np.sqrt(K)

TOK_TILES = W // 128    # 8 token tiles per window
C_CHUNKS = C // 128     # 4
HD_TILES = HK // 128    # 2
Q_TILES = W // 512      # 2 query tiles per window
S_CHUNKS = W // 128     # 8 key chunks per window
HPAIRS = H // 2         # 4 head pairs
EX_BUFS = 5
LN_BUFS = 8
ZW_BUFS = 6
OUTP_BUFS = 6


def _build_program(reps=1, do_ln=True, do_qkv=True, do_attn=True, do_av=True,
                   do_out=True, do_exp=True, ln_dram_bounce=True, alt_hp=False,
                   ln_fine_transpose=False, do_inject=True, early_start=True,
                   qk_c_outer=False, has_bo=False, ln_pool=True, ln_lnexp=True,
                   packed_norm=False, pipe=True, dve_exp_mod=0, pe_head_tp=False):
    nc = bacc.Bacc("TRN2", target_bir_lowering=False, debug=False)

    x_d = nc.dram_tensor("x", [WPC * W, C], F32, kind="ExternalInput")
    # q/k/v weights packed in one tensor (single DMA: the head is
    # DMA-issue-bound at ~650ns per HWDGE descriptor chain)
    wqkv_d = nc.dram_tensor(
        "wqkv", [3, C_CHUNKS, 128, HK], BF16, kind="ExternalInput"
    )
    wo_d = nc.dram_tensor("wo", [HD_TILES, 128, C], BF16, kind="ExternalInput")
    bqk_d = nc.dram_tensor("bqk", [128, 2 * HD_TILES], F32, kind="ExternalInput")
    ident_d = nc.dram_tensor("ident", [128, 128], BF16, kind="ExternalInput")
    bo_d = (
        nc.dram_tensor("bo", [1, C], F32, kind="ExternalInput")
        if has_bo
        else None
    )
    out_d = nc.dram_tensor("out", [WPC * W, C], F32, kind="ExternalOutput")
    z_d = nc.dram_tensor("z_scratch", [WPC, W, C], BF16)

    with tile.TileContext(nc) as tc:
        with (
            tc.tile_pool(name="const", bufs=1) as const,
            tc.tile_pool(name="xres", bufs=1) as xres,
            tc.tile_pool(name="zt", bufs=1) as ztp,
            tc.tile_pool(name="ln", bufs=LN_BUFS) as ln,
            tc.tile_pool(name="zw", bufs=ZW_BUFS) as zw,
            tc.tile_pool(name="qk", bufs=2) as qk,
            tc.tile_pool(name="vp", bufs=2) as vp,
            tc.tile_pool(name="ot", bufs=2) as otp,
            tc.tile_pool(name="ex", bufs=EX_BUFS) as ex,
            tc.tile_pool(name="tmp", bufs=8) as tmp,
            tc.tile_pool(name="outp", bufs=OUTP_BUFS) as outp,
            tc.tile_pool(name="ps_proj", bufs=2, space="PSUM") as ps_proj,
            tc.tile_pool(name="ps_sc", bufs=2, space="PSUM") as ps_sc_pool,
            tc.tile_pool(name="ps_acc", bufs=2, space="PSUM") as ps_acc,
        ):
            from contextlib import ExitStack as _ES
            _es = _ES()
            if reps > 1:
                _es.enter_context(
                    tc.For_i(
                        0, reps, 1,
                        hint_engines=(
                            mybir.EngineType.PE,
                            mybir.EngineType.Activation,
                            mybir.EngineType.DVE,
                            mybir.EngineType.SP,
                        ),
                    )
                )
            # ---- constants / weights (3 DMAs total) ----------------------
            eps_t = const.tile([128, 1], F32)
            w_all = const.tile([128, 3, C_CHUNKS, HK], BF16)
            wq_s = w_all[:, 0]
            wk_s = w_all[:, 1]
            wv_s = w_all[:, 2]
            wo_s = const.tile([128, HD_TILES, C], BF16)
            bqk_s = const.tile([128, 2 * HD_TILES], F32)
            bq_s = bqk_s[:, 0:HD_TILES]
            bk_s = bqk_s[:, HD_TILES : 2 * HD_TILES]
            bo_s = const.tile([128, C], F32) if has_bo else None
            ident_s = const.tile([128, 128], BF16)

            def const_thunk():
                nc.vector.memset(eps_t, EPS)
                if pe_head_tp:
                    nc.sync.dma_start(ident_s, ident_d[0:][:128, :])
                nc.sync.dma_start(
                    w_all,
                    bass.AP(
                        tensor=wqkv_d.ap().tensor,
                        offset=0,
                        ap=[[HK, 128], [C_CHUNKS * 128 * HK, 3],
                            [128 * HK, C_CHUNKS], [1, HK]],
                    ),
                )
                nc.sync.dma_start(
                    wo_s,
                    bass.AP(
                        tensor=wo_d.ap().tensor,
                        offset=0,
                        ap=[[C, 128], [128 * C, HD_TILES], [1, C]],
                    ),
                )
                nc.sync.dma_start(bqk_s, bqk_d[0:][:128, :])
                if has_bo:
                    # bo broadcast to all partitions (0-stride source)
                    nc.sync.dma_start(
                        bo_s,
                        bass.AP(
                            tensor=bo_d.ap().tensor,
                            offset=0,
                            ap=[[0, 128], [1, C]],
                        ),
                    )

            # ---- phase 1: LayerNorm + transpose (both windows) -----------
            xs = [
                [xres.tile([128, C], F32, name=f"x_{w}_{t}", tag=f"x_{w}_{t}")
                 for t in range(TOK_TILES)]
                for w in range(WPC)
            ]
            zT = [
                [ztp.tile([128, W], BF16, name=f"zT_{w}_{c}", tag=f"zT_{w}_{c}")
                 for c in range(C_CHUNKS)]
                for w in range(WPC)
            ]
            if not do_ln:
                for w in range(WPC):
                    for t in range(TOK_TILES):
                        nc.sync.dma_start(
                            xs[w][t], x_d[(w * TOK_TILES + t) * 128 :][:128, :]
                        )
                    for c in range(C_CHUNKS):
                        nc.gpsimd.memset(zT[w][c], 0.001)
            def ln_thunks(w, split_dma=False):
                """LN for window w as a thunk list (injectable), in two
                half-window batches of 4 token tiles for latency:
                [A0-3, B1, C0-3, D1, A4-7, B2, C4-7, D2] where A=dma+stats,
                B=batched rstd (DVE Newton rsqrt), C=z-write (GpSimd)+DMA,
                D=zT transposes for that token half. With split_dma, the x
                DMAs are returned separately (to pre-issue: an injected
                stats op whose DMA is still in flight blocks the whole DVE
                queue behind it)."""
                assert ln_lnexp and ln_dram_bounce and not ln_fine_transpose
                mvs = ln.tile([128, 2, TOK_TILES], F32, tag="mvs")
                rstds = ln.tile([128, TOK_TILES], F32, tag="rstds")

                def th_dma(t):
                    nc.sync.dma_start(
                        xs[w][t], x_d[(w * TOK_TILES + t) * 128 :][:128, :]
                    )

                def th_a(t):
                    if not split_dma:
                        th_dma(t)
                    stats = ln.tile([128, 6], F32, tag="stats")
                    nc.vector.bn_stats(out=stats, in_=xs[w][t])
                    nc.vector.bn_aggr(out=mvs[:, :, t : t + 1], in_=stats)

                def th_b(lo, hi):
                    # rstd = 1/sqrt(var+eps) on DVE (quake init + 2 Newton
                    # steps, ~1e-6 rel): keeps ScalarE exp-only so the
                    # program needs a single ACT table set load.
                    I32 = mybir.dt.int32
                    n = hi - lo
                    ve = ln.tile([128, n], F32, tag="ve")
                    nc.vector.tensor_scalar_add(out=ve, in0=mvs[:, 1, lo:hi],
                                                scalar1=float(EPS))
                    ti = ln.tile([128, n], I32, tag="ti")
                    nc.vector.tensor_scalar(
                        out=ti, in0=ve[:].bitcast(I32),
                        scalar1=1, scalar2=-1,
                        op0=mybir.AluOpType.logical_shift_right,
                        op1=mybir.AluOpType.bitwise_xor,
                    )
                    nc.vector.tensor_scalar_add(
                        out=ti, in0=ti, scalar1=0x5F3759DF + 1
                    )
                    y0 = ti[:].bitcast(F32)
                    t1 = ln.tile([128, n], F32, tag="t1")
                    t2 = ln.tile([128, n], F32, tag="t2")
                    nc.vector.tensor_mul(out=t1, in0=y0, in1=y0)
                    nc.vector.tensor_mul(out=t2, in0=t1, in1=ve)
                    nc.vector.tensor_scalar(
                        out=t1, in0=t2, scalar1=-0.5, scalar2=1.5,
                        op0=mybir.AluOpType.mult, op1=mybir.AluOpType.add,
                    )
                    nc.vector.tensor_mul(out=t2, in0=t1, in1=y0)  # y1
                    nc.vector.tensor_mul(out=t1, in0=t2, in1=t2)
                    nc.vector.tensor_mul(out=y0, in0=t1, in1=ve)
                    nc.vector.tensor_scalar(
                        out=t1, in0=y0, scalar1=-0.5, scalar2=1.5,
                        op0=mybir.AluOpType.mult, op1=mybir.AluOpType.add,
                    )
                    nc.vector.tensor_mul(out=rstds[:, lo:hi], in0=t1, in1=t2)

                z_sb = {}

                def th_c(t):
                    z_t = zw.tile([128, C], BF16, tag="z")
                    z_sb[t] = z_t
                    eng = nc.gpsimd if ln_pool else nc.vector
                    eng.tensor_scalar(
                        out=z_t,
                        in0=xs[w][t],
                        scalar1=mvs[:, 0, t : t + 1],
                        scalar2=rstds[:, t : t + 1],
                        op0=mybir.AluOpType.subtract,
                        op1=mybir.AluOpType.mult,
                    )
                    if not (pe_head_tp and w == 0 and t < TOK_TILES // 2):
                        nc.gpsimd.dma_start(z_d[w, t * 128 :][:128, :], z_t)

                def th_d(lo, hi):
                    if pe_head_tp and w == 0 and lo == 0:
                        # head is DMA-latency-bound; PE is idle -> transpose
                        # the first half-window on the PE via identity matmul
                        for t in range(lo, hi):
                            tp = ps_proj.tile(
                                [128, C], BF16, name="ps_p", tag="proj"
                            )
                            for c in range(C_CHUNKS):
                                nc.tensor.transpose(
                                    tp[:, c * 128 : (c + 1) * 128],
                                    z_sb[t][:, c * 128 : (c + 1) * 128],
                                    ident_s,
                                )
                            for c in range(C_CHUNKS):
                                nc.vector.tensor_copy(
                                    zT[w][c][:, t * 128 : (t + 1) * 128],
                                    tp[:, c * 128 : (c + 1) * 128],
                                )
                        return
                    for c in range(C_CHUNKS):
                        nc.sync.dma_start(
                            zT[w][c][:, lo * 128 : hi * 128],
                            z_d[w][lo * 128 : hi * 128,
                                   c * 128 : (c + 1) * 128],
                            transpose=True,
                        )

                H2 = TOK_TILES // 2
                ths = []
                for t in range(H2):
                    ths.append(lambda t=t: th_a(t))
                ths.append(lambda: th_b(0, H2))
                for t in range(H2):
                    ths.append(lambda t=t: th_c(t))
                ths.append(lambda: th_d(0, H2))
                for t in range(H2, TOK_TILES):
                    ths.append(lambda t=t: th_a(t))
                ths.append(lambda: th_b(H2, TOK_TILES))
                for t in range(H2, TOK_TILES):
                    ths.append(lambda t=t: th_c(t))
                ths.append(lambda: th_d(H2, TOK_TILES))
                if split_dma:
                    dmas = [
                        (lambda t=t: th_dma(t)) for t in range(TOK_TILES)
                    ]
                    return dmas, ths
                return ths

            _head_pipe = (
                do_ln and pipe and early_start and do_qkv and do_attn
            )
            if do_ln:
                lnw0 = ln_thunks(0)
                if _head_pipe:
                    # x DMAs first (the SP issue queue is the head
                    # bottleneck), then the packed const DMAs
                    for th in lnw0[:4]:
                        th()
                    const_thunk()
                else:
                    const_thunk()
                    for th in lnw0:
                        th()
                    if not pipe:
                        for th in ln_thunks(1):
                            th()
            else:
                const_thunk()

            # ---- phase 2: both windows, proj work injected into attention ---
            # Allocate per-window destination tiles eagerly (slot assignment
            # only; writes are emitted later by thunks).
            qkt = {}
            vs = {}
            oTs = {}
            vs_memsets = []
            for w in range(WPC):
                for name in ("q", "k"):
                    for m in range(HD_TILES):
                        qkt[(w, name, m)] = qk.tile(
                            [128, W], BF16,
                            name=f"{name}T_{w}_{m}", tag=f"{name}T_{m}",
                        )
                if packed_norm:
                    # per (t, u-headpair) 128-col block: [1 | V_2u | V_2u+1 | 1]
                    # -> AV psum rows: [den_h0, num_h0, num_h1, den_h1]; the
                    # numerators land contiguous (32:96) so ONE tensor_mul
                    # normalizes both heads.
                    v_s = vp.tile(
                        [128, TOK_TILES, 4, 4, 32], BF16, name=f"v_{w}", tag="v"
                    )
                    vs[w] = v_s
                    vs_memsets.append(
                        lambda v_s=v_s: (
                            nc.gpsimd.memset(v_s[:, :, :, 0, :], 1.0),
                            nc.gpsimd.memset(v_s[:, :, :, 3, :], 1.0),
                        )
                    )
                else:
                    v_s = vp.tile(
                        [128, TOK_TILES, H, 64], BF16, name=f"v_{w}", tag="v"
                    )
                    vs[w] = v_s
                    vs_memsets.append(
                        lambda v_s=v_s: nc.gpsimd.memset(
                            v_s[:, :, :, 32:64], 1.0
                        )
                    )
                oTs[w] = [
                    otp.tile([128, W], BF16, name=f"oT_{w}_{g}", tag=f"oT_{g}")
                    for g in range(HD_TILES)
                ]

            def qkv_thunks(w):
                """One thunk per PSUM group of the QKV projections."""
                if not do_qkv:
                    def th_init():
                        for name in ("q", "k"):
                            for m in range(HD_TILES):
                                nc.gpsimd.memset(qkt[(w, name, m)], 0.001)
                        if packed_norm:
                            nc.gpsimd.memset(vs[w][:, :, :, 1:3, :], 0.001)
                        else:
                            nc.gpsimd.memset(vs[w][:, :, :, 0:32], 0.001)
                    return [th_init]
                ths = []
                for name, w_s, b_s in (("q", wq_s, bq_s), ("k", wk_s, bk_s)):
                    for m in range(HD_TILES):
                        if qk_c_outer:
                            # c outer: each weight chunk loaded once, used
                            # for both n-tiles (two PSUM tiles live).
                            def th(name=name, w_s=w_s, b_s=b_s, m=m):
                                dst = qkt[(w, name, m)]
                                pss = [
                                    ps_proj.tile(
                                        [128, 512], F32, name="ps_p", tag="proj"
                                    )
                                    for _ in range(Q_TILES)
                                ]
                                for c in range(C_CHUNKS):
                                    for n in range(Q_TILES):
                                        nc.tensor.matmul(
                                            pss[n],
                                            lhsT=w_s[:, c, m * 128 : (m + 1) * 128],
                                            rhs=zT[w][c][:, n * 512 : (n + 1) * 512],
                                            start=(c == 0),
                                            stop=(c == C_CHUNKS - 1),
                                        )
                                for n in range(Q_TILES):
                                    nc.vector.tensor_scalar_add(
                                        out=dst[:, n * 512 : (n + 1) * 512],
                                        in0=pss[n],
                                        scalar1=b_s[:, m : m + 1],
                                    )
                            ths.append(th)
                            continue
                        for n in range(Q_TILES):
                            def th(name=name, w_s=w_s, b_s=b_s, m=m, n=n):
                                dst = qkt[(w, name, m)]
                                ps = ps_proj.tile(
                                    [128, 512], F32, name="ps_p", tag="proj"
                                )
                                for c in range(C_CHUNKS):
                                    nc.tensor.matmul(
                                        ps,
                                        lhsT=w_s[:, c, m * 128 : (m + 1) * 128],
                                        rhs=zT[w][c][:, n * 512 : (n + 1) * 512],
                                        start=(c == 0),
                                        stop=(c == C_CHUNKS - 1),
                                    )
                                nc.vector.tensor_scalar_add(
                                    out=dst[:, n * 512 : (n + 1) * 512],
                                    in0=ps,
                                    scalar1=b_s[:, m : m + 1],
                                )
                            ths.append(th)
                for t in range(TOK_TILES):
                    def th(t=t):
                        ps = ps_proj.tile([128, 512], F32, name="ps_p", tag="proj")
                        psv = ps[:, :HK]
                        for c in range(C_CHUNKS):
                            nc.tensor.matmul(
                                psv,
                                lhsT=zT[w][c][:, t * 128 : (t + 1) * 128],
                                rhs=wv_s[:, c, :],
                                start=(c == 0),
                                stop=(c == C_CHUNKS - 1),
                            )
                        if packed_norm:
                            nc.vector.tensor_copy(vs[w][:, t, :, 1:3, :], psv)
                        else:
                            nc.vector.tensor_copy(
                                vs[w][:, t, :, 0:32],
                                psv.rearrange("p (h k) -> p h k", h=H),
                            )
                    ths.append(th)
                return ths

            def outproj_thunk(w, t):
                def th():
                    oT = oTs[w]
                    ps = ps_proj.tile([128, 512], F32, name="ps_p", tag="proj")
                    for g in range(HD_TILES):
                        nc.tensor.matmul(
                            ps,
                            lhsT=oT[g][:, t * 128 : (t + 1) * 128],
                            rhs=wo_s[:, g, :],
                            start=(g == 0),
                            stop=(g == HD_TILES - 1),
                        )
                    o_t = outp.tile([128, C], F32, tag="o")
                    nc.vector.tensor_add(out=o_t, in0=ps, in1=xs[w][t])
                    if has_bo:
                        nc.vector.tensor_add(out=o_t, in0=o_t, in1=bo_s)
                    nc.sync.dma_start(
                        out_d[(w * TOK_TILES + t) * 128 :][:128, :], o_t
                    )
                return th

            def attn_emit(w, inject):
                """inject: dict {iter_index: [thunks]} emitted inside the
                pipeline (between the lookahead scores and this iteration's
                A*V) to fill PE gaps."""
                oT = oTs[w]
                if not (do_attn and do_av):
                    for g in range(HD_TILES):
                        nc.gpsimd.memset(oT[g], 0.001)
                if not do_attn:
                    for i in sorted(inject):
                        for th in inject[i]:
                            th()
                    return
                # head pairs alternate (hp even/odd) so consecutive
                # iterations touch different PE row halves (LDW pull-ahead).
                if alt_hp:
                    iters = [
                        (qt, 2 * hpp + sub, cch)
                        for qt in range(Q_TILES)
                        for hpp in range(HPAIRS // 2)
                        for cch in range(S_CHUNKS)
                        for sub in range(2)
                    ]
                else:
                    iters = [
                        (qt, hp, cch)
                        for qt in range(Q_TILES)
                        for hp in range(HPAIRS)
                        for cch in range(S_CHUNKS)
                    ]

                def emit_scores(qt, hp, cch):
                    ps_sc = ps_sc_pool.tile([128, 1024], F32, name="ps_sc", tag="sc")
                    hg = hp // 2
                    for j in range(2):
                        h = 2 * hp + j
                        g = h - 4 * hg
                        nc.tensor.matmul(
                            ps_sc[:, j * 512 : (j + 1) * 512],
                            lhsT=qkt[(w, "k", hg)][
                                g * 32 : (g + 1) * 32,
                                cch * 128 : (cch + 1) * 128,
                            ],
                            rhs=qkt[(w, "q", hg)][
                                g * 32 : (g + 1) * 32,
                                qt * 512 : (qt + 1) * 512,
                            ],
                            tile_position=(g * 32, 0),
                        )
                    return ps_sc

                accs = {}
                ps_sc = emit_scores(*iters[0])
                for i, (qt, hp, cch) in enumerate(iters):
                    if cch == 0:
                        accs[hp] = ps_acc.tile([128, 512], F32, name="ps_av", tag="acc")
                    acc = accs[hp]
                    expT = ex.tile([128, 1024], BF16, name="expT", tag="exp")
                    if do_exp:
                        if dve_exp_mod and i % dve_exp_mod == dve_exp_mod - 1:
                            # Schraudolph exp on DVE (offload from ScalarE):
                            # bf16 bits = round(x*scale*log2e*128 + (16256-C))
                            # computed as int16, aliased as bf16. Max exp rel
                            # err ~3.3% with C=5.5; softmax normalization
                            # cancels most of it (validated 2.3e-3 end-to-end
                            # with ALL tiles offloaded).
                            nc.vector.tensor_scalar(
                                out=expT[:].bitcast(mybir.dt.int16),
                                in0=ps_sc,
                                scalar1=float(1.4426950408889634 * 128.0
                                              * SCALE),
                                scalar2=float(127.0 * 128.0 - 5.5),
                                op0=mybir.AluOpType.mult,
                                op1=mybir.AluOpType.add,
                            )
                        else:
                            nc.scalar.activation(
                                out=expT, in_=ps_sc,
                                func=mybir.ActivationFunctionType.Exp,
                                scale=float(SCALE),
                            )
                    elif do_av:
                        nc.vector.memset(expT, 0.001)
                    # next iteration's scores BEFORE this iteration's AV
                    if i + 1 < len(iters):
                        ps_sc = emit_scores(*iters[i + 1])
                    for th in inject.get(i, ()):
                        th()
                    for j in range(2 if do_av else 0):
                        h = 2 * hp + j
                        if packed_norm:
                            lhsT = vs[w][:, cch, hp % (H // 2), 2 * j : 2 * j + 2, :]
                        else:
                            lhsT = vs[w][:, cch, h, :]
                        nc.tensor.matmul(
                            acc[j * 64 : (j + 1) * 64, :],
                            lhsT=lhsT,
                            rhs=expT[:, j * 512 : (j + 1) * 512],
                            start=(cch == 0),
                            stop=(cch == S_CHUNKS - 1),
                            tile_position=(0, j * 64),
                            skip_group_check=True,
                        )
                    if do_av and cch == S_CHUNKS - 1 and packed_norm:
                        # acc rows: [den_h0 | num_h0 | num_h1 | den_h1]
                        hg = hp // 2
                        g0 = 2 * (hp % 2)
                        rec = tmp.tile([64, 512], F32, tag="rec")
                        nc.vector.reciprocal(out=rec[0:32, :], in_=acc[0:32, :])
                        nc.vector.reciprocal(
                            out=rec[32:64, :], in_=acc[96:128, :]
                        )
                        nc.vector.tensor_mul(
                            out=oT[hg][
                                g0 * 32 : g0 * 32 + 64,
                                qt * 512 : (qt + 1) * 512,
                            ],
                            in0=acc[32:96, :],
                            in1=rec,
                        )
                    elif do_av and cch == S_CHUNKS - 1:
                        for j in range(2):
                            h = 2 * hp + j
                            hg = h // 4
                            g = h - 4 * hg
                            rec = tmp.tile([32, 512], F32, tag=f"rec{j}")
                            nc.vector.reciprocal(
                                out=rec, in_=acc[j * 64 + 32 : j * 64 + 64, :]
                            )
                            nc.vector.tensor_mul(
                                out=oT[hg][
                                    g * 32 : (g + 1) * 32,
                                    qt * 512 : (qt + 1) * 512,
                                ],
                                in0=acc[j * 64 : j * 64 + 32, :],
                                in1=rec,
                            )
                for i in sorted(inject):
                    if i >= len(iters):
                        for th in inject[i]:
                            th()

            n_iters = Q_TILES * HPAIRS * S_CHUNKS

            # window 0 QKV: either up front, or dependency-ordered and
            # injected into the first attention iterations (early start)
            ths_w0 = qkv_thunks(0)
            if early_start and do_qkv and do_attn:
                # [K00, Q00, V0] up front; rest injected at iters 0..12 in
                # deadline order (inject at iter i is visible to the scores
                # lookahead of iter i+2 and to AV of iter i): V_t by idx t,
                # K(0,1) by idx 2, K(1,*)/Q(1,0) by idx 14, Q(0,1) by 30.
                # qkv_thunks order: Q(0,0) Q(0,1) Q(1,0) Q(1,1)
                #                   K(0,0) K(0,1) K(1,0) K(1,1) V0..V7
                o = [4, 0, 8, 9, 5, 10, 11, 12, 13, 14, 15, 6, 2, 7, 1, 3]
                ths_w0 = [ths_w0[i] for i in o]
                if _head_pipe:
                    # rest of LN(w0) first half, then first projections
                    # (which only need the first zT halves), then the
                    # second LN(w0) half
                    for th in lnw0[4:10]:
                        th()
                    for th in ths_w0[:3]:
                        th()
                    for th in lnw0[10:]:
                        th()
                else:
                    for th in ths_w0[:3]:
                        th()
                inj_es = {}
                # ths_w0[3:] = [V1 K01 V2 V3 V4 V5 V6 V7 K10 Q10 K11 Q01 Q11]
                # spread by deadline so the per-iter PE surcharge stays small
                _slots = [0, 2, 1, 2, 3, 4, 5, 6, 9, 12, 15, 22, 30]
                for idx, th in enumerate(ths_w0[3:]):
                    inj_es.setdefault(_slots[idx], []).append(th)
            else:
                for th in ths_w0:
                    th()
                inj_es = {}
            inj0 = dict(inj_es)
            for th in vs_memsets:
                th()
            if pipe and do_ln and do_qkv and do_attn:
                # LN(w1) crammed early (DMA/DVE/Pool only — no PE cost),
                # QKV(w1) spread every 2 iters under attention(w0)
                lnw1_dma, lnw1 = ln_thunks(1, split_dma=True)
                for th in lnw1_dma:
                    th()
                for idx, th in enumerate(lnw1):
                    inj0.setdefault(10 + idx, []).append(th)
                ths_w1 = qkv_thunks(1)
                for idx, th in enumerate(ths_w1):
                    inj0.setdefault(min(31 + 2 * idx, n_iters - 1), []).append(th)
                attn_emit(0, inj0)
            else:
                ths_w1 = qkv_thunks(1)
                if do_inject:
                    step = max(1, n_iters // (len(ths_w1) + 1))
                    for idx, th in enumerate(ths_w1):
                        inj0.setdefault(
                            min((idx + 1) * step, n_iters - 1), []
                        ).append(th)
                    attn_emit(0, inj0)
                else:
                    attn_emit(0, inj0)
                    for th in ths_w1:
                        th()

            # attention(w1) with outproj(w0) injected early and
            # outproj(w1) for qt=0 tiles injected in the qt=1 half
            inj1 = {}
            if do_out and do_inject:
                for idx, t in enumerate(range(TOK_TILES)):
                    inj1.setdefault(min(4 + idx * 7, n_iters - 1), []).append(
                        outproj_thunk(0, t)
                    )
                half = n_iters // 2
                for idx, t in enumerate(range(TOK_TILES // 2)):
                    inj1.setdefault(
                        min(half + 4 + idx * 10, n_iters - 1), []
                    ).append(outproj_thunk(1, t))
                for idx, t in enumerate(range(TOK_TILES // 2, TOK_TILES)):
                    inj1.setdefault(n_iters + idx, []).append(outproj_thunk(1, t))
                attn_emit(1, inj1)
            else:
                if do_out:
                    for t in range(TOK_TILES):
                        outproj_thunk(0, t)()
                attn_emit(1, {})
                if do_out:
                    for t in range(TOK_TILES):
                        outproj_thunk(1, t)()

            _es.close()

    nc.compile()
    return nc


_CACHE = {}


def _get_program(has_bo):
    key = ("nc", has_bo)
    if key not in _CACHE:
        _CACHE[key] = _build_program(has_bo=has_bo)
    return _CACHE[key]


def _prep_inputs(x, ln_gamma, ln_beta, Wq, bq, Wk, bk, Wv, bv, Wo, bo):
    """Host-side constant folding + sharding. Returns per-core in_maps."""
    x = np.asarray(x, np.float32)
    g = np.asarray(ln_gamma, np.float32)
    be = np.asarray(ln_beta, np.float32)
    Wq = np.asarray(Wq, np.float32).reshape(C, HK)
    Wk = np.asarray(Wk, np.float32).reshape(C, HK)
    Wv = np.asarray(Wv, np.float32).reshape(C, HK)
    Wo2 = np.asarray(Wo, np.float32).reshape(HK, C)
    bq = np.asarray(bq, np.float32).reshape(HK)
    bk = np.asarray(bk, np.float32).reshape(HK)
    bv = np.asarray(bv, np.float32).reshape(HK)
    bo = np.asarray(bo, np.float32).reshape(C)

    # Fold LN affine (z = n*gamma + beta) into projections:
    #   z @ W + b = n @ (gamma[:,None]*W) + (beta @ W + b)
    Wq_e = g[:, None] * Wq
    Wk_e = g[:, None] * Wk
    Wv_e = g[:, None] * Wv
    bq_e = be @ Wq + bq
    bk_e = be @ Wk + bk
    bv_e = be @ Wv + bv
    # Softmax rows sum to 1 -> value bias passes through attention:
    #   attn @ (V + 1 bv) @ Wo + bo = attn @ V @ Wo + (bv @ Wo + bo)
    bo_e = bv_e @ Wo2 + bo

    bf = ml_dtypes.bfloat16
    wqkv_h = np.ascontiguousarray(
        np.stack([Wq_e, Wk_e, Wv_e]).reshape(3, C_CHUNKS, 128, HK)
    ).astype(bf)
    wo_h = Wo2.reshape(HD_TILES, 128, C).astype(bf)
    bqk_h = np.ascontiguousarray(
        np.stack(
            [bq_e[0:128], bq_e[128:256], bk_e[0:128], bk_e[128:256]], axis=1
        )
    ).astype(np.float32)
    bo_h = bo_e.reshape(1, C).astype(np.float32)

    has_bo = bool(np.any(bo_e != 0))
    xw = np.ascontiguousarray(x.reshape(NW, W, C))
    in_maps = []
    for i in range(N_CORES):
        shard = np.ascontiguousarray(
            xw[i * WPC : (i + 1) * WPC].reshape(WPC * W, C)
        )
        m = {
            "x": shard,
            "wqkv": wqkv_h, "wo": wo_h, "bqk": bqk_h,
            "ident": np.eye(128, dtype=bf),
        }
        if has_bo:
            m["bo"] = bo_h
        in_maps.append(m)
    return in_maps


def kernel(x, ln_gamma, ln_beta, Wq, bq, Wk, bk, Wv, bv, Wo, bo):
    in_maps = _prep_inputs(x, ln_gamma, ln_beta, Wq, bq, Wk, bk, Wv, bv, Wo, bo)
    nc = _get_program(has_bo="bo" in in_maps[0])
    res = run_bass_kernel_spmd(nc, in_maps, core_ids=list(range(N_CORES)))
    out = np.concatenate([res.results[i]["out"] for i in range(N_CORES)], axis=0)
    return np.ascontiguousarray(out.reshape(B, T, C)).astype(np.float32)

